# revision 1
# baseline (speedup 1.0000x reference)
"""Cross-attention Trainium2 kernel (8 NeuronCores, batch-data-parallel).

Computes, per batch element b:
    q = x[b] @ Wq            [S, DK]
    k = y[b] @ Wk            [S, DK]
    v = y[b] @ Wv            [S, E]
    p = exp((q @ k.T) / sqrt(E))        (no max-subtraction: logits ~ N(0, .25))
    out[b] = (p @ v) / rowsum(p) + x[b]

Layout strategy (per core, BL=2 batches):
  - Everything on TensorE is bf16 with fp32 PSUM accumulation.
  - Activations are transposed on-chip (cast-DMA fp32->bf16 into a DRAM
    bounce, then xbar DMA-transpose into SBUF) so the contraction dim of
    every matmul sits on partitions:
        xT, yT : [C, S]     qT = Wq.T @ xT : [DK, S]   kT : [DK, S]
        v  = yT.T @ Wv : [S_kv, E]  (natural layout)
        sT = kT.T @ qT : [S_kv, S_q]   (scoresT; softmax axis = partitions)
        pT = exp(sT/32)                (stationary of the AV matmul)
        out = pT.T @ [v | 1]           (ones column yields rowsum(p) free)
  - Epilogue fuses (psum * 1/rowsum) + x in one DVE scalar_tensor_tensor.
"""

import math

import numpy as np

# Full-problem constants (hardcoded per the harness contract).
B_FULL = 16
N_CORES = 8
S_Q = 2048
S_KV = 2048
C_DIM = 1024  # input feature dim (contraction of the projections)
DK = 256  # q/k head dim
E_DIM = 1024  # v / output dim
P = 128


class CFG:
    def __init__(self, bl, sq, skv, c, dk, e, s_block=None, n_free=512):
        assert sq % P == 0 and skv % P == 0 and c % P == 0 and dk % P == 0
        self.bl = bl  # batches per core
        self.sq = sq
        self.skv = skv
        self.c = c
        self.dk = dk
        self.e = e
        self.s_block = s_block or min(1024, sq)  # query cols processed per wave
        assert sq % self.s_block == 0
        self.n_free = n_free  # moving-operand free-dim per matmul
        self.scale = 1.0 / math.sqrt(e)


def _chunks(total, size):
    out = []
    o = 0
    while o < total:
        out.append((o, min(size, total - o)))
        o += size
    return out


def emit_cross_attention(tc, outs, ins, cfg):
    """Emit the kernel into TileContext `tc`.

    outs/ins are dicts of DRAM APs: ins = x, y, Wq, Wk, Wv ; outs = out.
    x/y/out: [bl, sq|skv, c|e] fp32. Weights: [c, dk|e] fp32.
    """
    import concourse.mybir as mybir
    from concourse.mybir import ActivationFunctionType as AF
    from concourse.mybir import AluOpType as ALU
    from concourse.tile_rust import add_dep_helper

    nc = tc.nc
    bf16 = mybir.dt.bfloat16
    f32 = mybir.dt.float32

    x, y, Wq, Wk, Wv = ins["x"], ins["y"], ins["Wq"], ins["Wk"], ins["Wv"]
    out = outs["out"]

    n_ct = cfg.c // P  # contraction tiles of the projections
    n_tt = cfg.skv // P  # key/value tiles (AV contraction)
    n_dt = cfg.dk // P  # qk-dim tiles (score contraction)
    s_waves = _chunks(cfg.sq, cfg.s_block)

    # DRAM bounce buffers for the bf16 copies of x and y (per local batch).
    xb = nc.dram_tensor("xb16", [cfg.bl, cfg.sq, cfg.c], bf16).ap()
    yb = nc.dram_tensor("yb16", [cfg.bl, cfg.skv, cfg.c], bf16).ap()

    pool = tc.alloc_tile_pool(name="main", bufs=1)
    ps_mm = tc.alloc_tile_pool(name="ps_mm", bufs=2, space="PSUM")
    ps_av = tc.alloc_tile_pool(name="ps_av", bufs=2, space="PSUM")

    # Measured DMA facts this layout is built on:
    #   - SWDGE D2D cast runs at ~360 GB/s payload.
    #   - xbar transposes cost ~1us fixed + ~400 GB/s; they only exist on
    #     one ring (concurrent transposes on both HWDGE rings corrupt), and
    #     Tile serializes every transpose group against ALL in-flight DMAs.
    #     So the global stream alternates copy-windows and transpose-windows,
    #     ordered here so each window's data is needed just after it closes.
    #   - SWDGE queue: casts only; sync ring: transposes only; scalar ring:
    #     weights / residual / output plain DMAs.
    half = cfg.skv // 2 if cfg.skv >= 1024 else cfg.skv
    y0_chunks = _chunks(cfg.skv, half)

    wq_sb = []
    wk_sb = []
    wv_sb = []
    for kc in range(n_ct):
        wq_t = pool.tile([P, cfg.dk], bf16, tag=f"wq{kc}", name=f"wq{kc}")
        wk_t = pool.tile([P, cfg.dk], bf16, tag=f"wk{kc}", name=f"wk{kc}")
        wv_t = pool.tile([P, cfg.e], bf16, tag=f"wv{kc}", name=f"wv{kc}")
        wq_sb.append(wq_t)
        wk_sb.append(wk_t)
        wv_sb.append(wv_t)

    def load_weight(which, w_dram, w_tiles, wdim, kc):
        w_f = pool.tile([P, cfg.e], f32, tag="wstage", bufs=2, name=f"wf{which}{kc}")
        nc.scalar.dma_start(out=w_f[:, :wdim], in_=w_dram[kc * P : (kc + 1) * P, :])
        nc.vector.tensor_copy(w_tiles[kc][:], w_f[:, :wdim])

    # copy-window 0: y0 first half cast (SWDGE) + wk loads (scalar ring)
    nc.gpsimd.dma_start(out=yb[0][0:half, :], in_=y[0][0:half, :])
    for kc in range(n_ct):
        load_weight("k", Wk, wk_sb, cfg.dk, kc)

    ones_col = pool.tile([P, 1], bf16, tag="ones", name="ones")
    nc.gpsimd.memset(ones_col[:], 1.0)

    allT = {}
    for b in range(cfg.bl):
        yT = []
        xT = []
        for kc in range(n_ct):
            yT_t = pool.tile([P, cfg.skv], bf16, tag="actT", bufs=2 * n_ct, name=f"yT{kc}")
            yT.append(yT_t)
        for kc in range(n_ct):
            xT_t = pool.tile([P, cfg.sq], bf16, tag="actT", bufs=2 * n_ct, name=f"xT{kc}")
            xT.append(xT_t)
        allT[b] = (yT, xT)

    def transpose_group(b, which, ro, rn):
        srcb = yb if which == "y" else xb
        dst = allT[b][0] if which == "y" else allT[b][1]
        last = None
        for kc in range(n_ct):
            last = nc.sync.dma_start(
                out=dst[kc][:, ro : ro + rn],
                in_=srcb[b][ro : ro + rn, kc * P : (kc + 1) * P],
                transpose=True,
            )
        return last

    def pace(waiter, dependee):
        # Real semaphore edge: keeps the next copy-window out of flight until
        # the previous transpose-window drains (Tile serializes any transpose
        # against every in-flight copy, so un-paced casts stall transposes).
        if waiter is not None and dependee is not None:
            add_dep_helper(waiter.ins, dependee.ins, sync=True, reason="pace dma windows")

    # transpose-window: yT(b0) first half
    tg = transpose_group(0, "y", 0, half)
    # copy-window: y0 second half + wv, wq loads
    if half < cfg.skv:
        c = nc.gpsimd.dma_start(out=yb[0][half:, :], in_=y[0][half:, :])
        pace(c, tg)
    for kc in range(n_ct):
        load_weight("v", Wv, wv_sb, cfg.e, kc)
    for kc in range(n_ct):
        load_weight("q", Wq, wq_sb, cfg.dk, kc)
    if half < cfg.skv:
        tg = transpose_group(0, "y", half, cfg.skv - half)
    # copy-window: x0 cast; then xT(b0) transposes
    c = nc.gpsimd.dma_start(out=xb[0][:], in_=x[0][:])
    pace(c, tg)
    tg = transpose_group(0, "x", 0, cfg.sq)
    allT["last_tg"] = tg
    # b1 chains are emitted inside the batch loop below (their windows land
    # under b0's scores/AV compute).

    for b in range(cfg.bl):
        yT, xT = allT[b]
        if b > 0:
            c = nc.gpsimd.dma_start(out=yb[b][:], in_=y[b][:])
            pace(c, allT["last_tg"])
            tg = transpose_group(b, "y", 0, cfg.skv)
            c = nc.gpsimd.dma_start(out=xb[b][:], in_=x[b][:])
            pace(c, tg)
            pace(c, allT.get(f"wave_end_{b - 1}_0"))
            tg = transpose_group(b, "x", 0, cfg.sq)
            allT["last_tg"] = tg

        # --- projections: kT/v aligned to the y halves, then qT ------------
        kT = []
        qT = []
        for md in range(n_dt):
            kT_t = pool.tile([P, cfg.skv], bf16, tag=f"kT{md}", name=f"kT{md}")
            qT_t = pool.tile([P, cfg.sq], bf16, tag=f"qT{md}", name=f"qT{md}")
            kT.append(kT_t)
            qT.append(qT_t)
        v_sb = [
            pool.tile([P, cfg.e], bf16, tag="v", bufs=n_tt, name=f"v{mt}")
            for mt in range(n_tt)
        ]

        for ro, rn in y0_chunks:
            for no, nn_ in _chunks(rn, cfg.n_free):
                for md in range(n_dt):
                    ps = ps_mm.tile([P, cfg.n_free], f32, tag="mm", name="ps_p")
                    for kc in range(n_ct):
                        nc.tensor.matmul(
                            ps[:, :nn_],
                            wk_sb[kc][:, md * P : (md + 1) * P],
                            yT[kc][:, ro + no : ro + no + nn_],
                            start=(kc == 0),
                            stop=(kc == n_ct - 1),
                        )
                    nc.scalar.activation(
                        kT[md][:, ro + no : ro + no + nn_], ps[:, :nn_], AF.Copy
                    )
            for mt in range(ro // P, (ro + rn) // P):
                v_t = v_sb[mt]
                for no, nn_ in _chunks(cfg.e, cfg.n_free):
                    ps = ps_mm.tile([P, cfg.n_free], f32, tag="mm", name="ps_v")
                    for kc in range(n_ct):
                        nc.tensor.matmul(
                            ps[:, :nn_],
                            yT[kc][:, mt * P : (mt + 1) * P],
                            wv_sb[kc][:, no : no + nn_],
                            start=(kc == 0),
                            stop=(kc == n_ct - 1),
                        )
                    nc.scalar.activation(v_t[:, no : no + nn_], ps[:, :nn_], AF.Copy)
        for no, nn_ in _chunks(cfg.sq, cfg.n_free):
            for md in range(n_dt):
                ps = ps_mm.tile([P, cfg.n_free], f32, tag="mm", name="ps_q")
                for kc in range(n_ct):
                    nc.tensor.matmul(
                        ps[:, :nn_],
                        wq_sb[kc][:, md * P : (md + 1) * P],
                        xT[kc][:, no : no + nn_],
                        start=(kc == 0),
                        stop=(kc == n_ct - 1),
                    )
                nc.scalar.activation(qT[md][:, no : no + nn_], ps[:, :nn_], AF.Copy)

        # --- attention, one wave of s_block query columns at a time --------
        for wo, wn in s_waves:
            # scoresT + exp: pT[t, s_block]
            pT = []
            for t in range(n_tt):
                pT_t = pool.tile([P, cfg.s_block], bf16, tag="pT", bufs=n_tt, name=f"pT{t}")
                for no, nn_ in _chunks(wn, cfg.n_free):
                    ps = ps_mm.tile([P, cfg.n_free], f32, tag="mm", name="ps_s")
                    for kd in range(n_dt):
                        nc.tensor.matmul(
                            ps[:, :nn_],
                            kT[kd][:, t * P : (t + 1) * P],
                            qT[kd][:, wo + no : wo + no + nn_],
                            start=(kd == 0),
                            stop=(kd == n_dt - 1),
                        )
                    nc.scalar.activation(
                        pT_t[:, no : no + nn_], ps[:, :nn_], AF.Exp, scale=cfg.scale
                    )
                pT.append(pT_t)

            # AV + rowsum + epilogue, per 128-row block of queries
            for mh in range(wn // P):
                sm = wo + mh * P  # global query row offset
                ps_e = ps_av.tile([P, cfg.e], f32, tag="av_e", name="ps_e")
                ps_sum = ps_av.tile([P, 1], f32, tag="av_s", name="ps_sum")
                e_chunks = _chunks(cfg.e, cfg.n_free)
                for t in range(n_tt):
                    lhsT = pT[t][:, mh * P : (mh + 1) * P]
                    for no, nn_ in e_chunks:
                        nc.tensor.matmul(
                            ps_e[:, no : no + nn_],
                            lhsT,
                            v_sb[t][:, no : no + nn_],
                            start=(t == 0),
                            stop=(t == n_tt - 1),
                        )
                    nc.tensor.matmul(
                        ps_sum[:],
                        lhsT,
                        ones_col[:],
                        start=(t == 0),
                        stop=(t == n_tt - 1),
                    )
                recip = pool.tile([P, 1], f32, tag="recip", bufs=4, name="recip")
                nc.vector.reciprocal(recip[:], ps_sum[:])
                xres = pool.tile([P, cfg.e], f32, tag="xres", bufs=3, name="xres")
                nc.scalar.dma_start(out=xres[:], in_=x[b][sm : sm + P, :])
                out_t = pool.tile([P, cfg.e], f32, tag="out_t", bufs=4, name="out_t")
                nc.vector.scalar_tensor_tensor(
                    out_t[:], ps_e[:], recip[:], xres[:], ALU.mult, ALU.add
                )
                st = nc.scalar.dma_start(out=out[b][sm : sm + P, :], in_=out_t[:])
                allT[f"wave_end_{b}_{wo}"] = st

    ps_av.release()
    ps_mm.release()
    pool.release()


def make_tile_kernel(cfg):
    """Adapter with the (tc, outs, ins) signature used by run_kernel/test.py."""

    def k(tc, outs, ins):
        emit_cross_attention(tc, outs, ins, cfg)

    return k


def _build(cfg):
    import concourse.bacc as bacc
    import concourse.mybir as mybir
    import concourse.tile as tile

    f32 = mybir.dt.float32
    nc = bacc.Bacc(
        "TRN2",
        target_bir_lowering=False,
        debug=False,
        enable_asserts=False,
        num_devices=N_CORES,
    )
    ins = {
        "x": nc.dram_tensor("x", [cfg.bl, cfg.sq, cfg.c], f32, kind="ExternalInput").ap(),
        "y": nc.dram_tensor("y", [cfg.bl, cfg.skv, cfg.c], f32, kind="ExternalInput").ap(),
        "Wq": nc.dram_tensor("Wq", [cfg.c, cfg.dk], f32, kind="ExternalInput").ap(),
        "Wk": nc.dram_tensor("Wk", [cfg.c, cfg.dk], f32, kind="ExternalInput").ap(),
        "Wv": nc.dram_tensor("Wv", [cfg.c, cfg.e], f32, kind="ExternalInput").ap(),
    }
    outs = {
        "out": nc.dram_tensor("out", [cfg.bl, cfg.sq, cfg.e], f32, kind="ExternalOutput").ap()
    }
    with tile.TileContext(nc) as tc:
        emit_cross_attention(tc, outs, ins, cfg)
    nc.compile()
    return nc


_CACHED = {}


def run_on_cores(x, y, Wq, Wk, Wv, trace=False):
    from concourse import bass_utils

    cfg = CFG(B_FULL // N_CORES, S_Q, S_KV, C_DIM, DK, E_DIM)
    key = "full"
    if key not in _CACHED:
        _CACHED[key] = _build(cfg)
    nc = _CACHED[key]

    bl = cfg.bl
    in_maps = [
        {
            "x": np.ascontiguousarray(x[i * bl : (i + 1) * bl]),
            "y": np.ascontiguousarray(y[i * bl : (i + 1) * bl]),
            "Wq": Wq,
            "Wk": Wk,
            "Wv": Wv,
        }
        for i in range(N_CORES)
    ]
    res = bass_utils.run_bass_kernel_spmd(
        nc, in_maps, core_ids=list(range(N_CORES)), trace=trace
    )
    out = np.concatenate([r["out"] for r in res.results], axis=0)
    return out, res


def kernel(x, y, Wq, Wk, Wv):
    x = np.asarray(x, dtype=np.float32)
    y = np.asarray(y, dtype=np.float32)
    Wq = np.asarray(Wq, dtype=np.float32)
    Wk = np.asarray(Wk, dtype=np.float32)
    Wv = np.asarray(Wv, dtype=np.float32)
    out, _ = run_on_cores(x, y, Wq, Wk, Wv, trace=False)
    return out



# revision 7
# speedup vs baseline: 1.0919x; 1.0919x over previous
"""Cross-attention Trainium2 kernel (8 NeuronCores, batch-data-parallel).

Computes, per batch element b:
    q = x[b] @ Wq            [S, DK]
    k = y[b] @ Wk            [S, DK]
    v = y[b] @ Wv            [S, E]
    p = exp((q @ k.T) / sqrt(E))        (no max-subtraction: logits ~ N(0, .25))
    out[b] = (p @ v) / rowsum(p) + x[b]

Layout strategy (per core, BL=2 batches):
  - All matmuls run in fp8e4 with perf_mode=DoubleRow (2 fp8 weights/cell,
    2 MACs/cycle): operands are stored as [128, 2, free] "k-pair" tiles so a
    single matmul contracts 256 elements; PSUM accumulates fp32.
  - Weights are pre-scaled by 8 when cast to fp8 (keeps N(0,1/1024) entries
    out of the fp8 subnormal range); the score scale folds the 8*8 back out,
    and the rowsum ones-column is 8.0 so the softmax normalization of the
    8x-scaled v cancels exactly.
  - Activations are transposed on-chip (cast-DMA fp32->bf16 into a DRAM
    bounce, then xbar DMA-transpose into SBUF bf16, then DVE cast to the
    fp8 k-pair tiles) so the contraction dim of every matmul sits on
    partitions:
        xT, yT : [C, S]     qT = Wq.T @ xT : [DK, S]   kT : [DK, S]
        v  = yT.T @ Wv : [S_kv, E]  (natural layout)
        sT = kT.T @ qT : [S_kv, S_q]   (scoresT; softmax axis = partitions)
        pT = exp(sT*scale)             (stationary of the AV matmul)
        out = pT.T @ [v | 8]           (8s column yields 8*rowsum(p) free)
  - Epilogue fuses (psum * 1/rowsum8) + x in one DVE scalar_tensor_tensor.
"""

import math

import numpy as np

# Full-problem constants (hardcoded per the harness contract).
B_FULL = 16
N_CORES = 8
S_Q = 2048
S_KV = 2048
C_DIM = 1024  # input feature dim (contraction of the projections)
DK = 256  # q/k head dim
E_DIM = 1024  # v / output dim
P = 128
WSC = 8.0  # fp8 pre-scale on Wq/Wk/Wv (and the rowsum ones column)


class CFG:
    def __init__(self, bl, sq, skv, c, dk, e, s_block=None, n_free=512):
        assert sq % P == 0 and skv % P == 0 and c % P == 0 and dk % P == 0
        self.bl = bl  # batches per core
        self.sq = sq
        self.skv = skv
        self.c = c
        self.dk = dk
        self.e = e
        self.s_block = s_block or min(1024, sq)  # query cols processed per wave
        assert sq % self.s_block == 0
        self.n_free = n_free  # moving-operand free-dim per matmul
        self.scale = 1.0 / math.sqrt(e)


def _chunks(total, size):
    out = []
    o = 0
    while o < total:
        out.append((o, min(size, total - o)))
        o += size
    return out


def emit_cross_attention(tc, outs, ins, cfg):
    """Emit the kernel into TileContext `tc`.

    outs/ins are dicts of DRAM APs: ins = x, y, Wq, Wk, Wv ; outs = out.
    x/y/out: [bl, sq|skv, c|e] fp32. Weights: [c, dk|e] fp32.
    """
    import concourse.mybir as mybir
    from concourse.mybir import ActivationFunctionType as AF
    from concourse.mybir import AluOpType as ALU
    from concourse.tile_rust import add_dep_helper

    DR = mybir.MatmulPerfMode.DoubleRow

    nc = tc.nc
    bf16 = mybir.dt.bfloat16
    fp8 = mybir.dt.float8e4
    f32 = mybir.dt.float32

    x, y, Wq, Wk, Wv = ins["x"], ins["y"], ins["Wq"], ins["Wk"], ins["Wv"]
    out = outs["out"]

    n_ct = cfg.c // P  # 128-contraction tiles of the projections
    n_cg = n_ct // 2  # DoubleRow (256-contraction) groups of the projections
    n_tt = cfg.skv // P  # key/value 128-tiles
    n_tg = n_tt // 2  # key/value DoubleRow groups (AV contraction)
    n_dt = cfg.dk // P  # qk-dim 128-tiles (score contraction; must be 2)
    assert n_dt == 2, "scores assume DK == 256 (one DoubleRow group)"
    s_waves = _chunks(cfg.sq, cfg.s_block)

    # DRAM bounce buffers for the bf16 copies of x and y (per local batch).
    xb = nc.dram_tensor("xb16", [cfg.bl, cfg.sq, cfg.c], bf16).ap()
    yb = nc.dram_tensor("yb16", [cfg.bl, cfg.skv, cfg.c], bf16).ap()

    pool = tc.alloc_tile_pool(name="main", bufs=1)
    ps_mm = tc.alloc_tile_pool(name="ps_mm", bufs=2, space="PSUM")
    ps_av = tc.alloc_tile_pool(name="ps_av", bufs=2, space="PSUM")

    # Measured DMA facts this layout is built on:
    #   - SWDGE D2D cast runs at ~360 GB/s payload.
    #   - xbar transposes cost ~1us fixed + ~400 GB/s; they only exist on
    #     one ring (concurrent transposes on both HWDGE rings corrupt), and
    #     Tile serializes every transpose group against ALL in-flight DMAs.
    #     So the global stream alternates copy-windows and transpose-windows,
    #     ordered here so each window's data is needed just after it closes.
    #   - SWDGE queue: casts only; sync ring: transposes only; scalar ring:
    #     weights / residual / output plain DMAs.
    half = cfg.skv // 2 if cfg.skv >= 1024 else cfg.skv
    y0_chunks = _chunks(cfg.skv, half)

    # fp8 k-pair weight tiles: w8[g][:, j, :] holds rows (2g+j)*128..+128.
    wq8 = [pool.tile([P, 2, cfg.dk], fp8, tag=f"wq{g}", name=f"wq{g}") for g in range(n_cg)]
    wk8 = [pool.tile([P, 2, cfg.dk], fp8, tag=f"wk{g}", name=f"wk{g}") for g in range(n_cg)]
    wv8 = [pool.tile([P, 2, cfg.e], fp8, tag=f"wv{g}", name=f"wv{g}") for g in range(n_cg)]

    def load_weight(which, w_dram, w_tiles, wdim, kc):
        w_f = pool.tile([P, cfg.e], f32, tag="wstage", bufs=2, name=f"wf{which}{kc}")
        nc.scalar.dma_start(out=w_f[:, :wdim], in_=w_dram[kc * P : (kc + 1) * P, :])
        nc.vector.tensor_scalar_mul(w_tiles[kc // 2][:, kc % 2, :], w_f[:, :wdim], WSC)

    # copy-window 0: ALL weight loads first (scalar ring) so no weight DMA is
    # in flight when the first transpose window opens (transposes serialize
    # against every in-flight DMA), then y0 first half cast (SWDGE).
    for kc in range(n_ct):
        load_weight("k", Wk, wk8, cfg.dk, kc)
    for kc in range(n_ct):
        load_weight("v", Wv, wv8, cfg.e, kc)
    for kc in range(n_ct):
        load_weight("q", Wq, wq8, cfg.dk, kc)
    nc.gpsimd.dma_start(out=yb[0][0:half, :], in_=y[0][0:half, :])

    # 8.0 column (fp8 exact): rowsum of p gets the same 8x scale as v.
    ones_col = pool.tile([P, 2, 16], fp8, tag="ones", name="ones")
    nc.gpsimd.memset(ones_col[:], WSC)

    allT = {}
    for b in range(cfg.bl):
        yT = []
        xT = []
        for kc in range(n_ct):
            yT_t = pool.tile([P, cfg.skv], bf16, tag="actT", bufs=2 * n_ct, name=f"yT{kc}")
            yT.append(yT_t)
        for kc in range(n_ct):
            xT_t = pool.tile([P, cfg.sq], bf16, tag="actT", bufs=2 * n_ct, name=f"xT{kc}")
            xT.append(xT_t)
        # fp8 k-pair copies of the transposed activations.
        yT8 = [
            pool.tile([P, 2, cfg.skv], fp8, tag="actT8", bufs=n_ct, name=f"yT8_{g}")
            for g in range(n_cg)
        ]
        xT8 = [
            pool.tile([P, 2, cfg.sq], fp8, tag="actT8", bufs=n_ct, name=f"xT8_{g}")
            for g in range(n_cg)
        ]
        allT[b] = (yT, xT, yT8, xT8)

    def transpose_group(b, which, ro, rn):
        srcb = yb if which == "y" else xb
        dst = allT[b][0] if which == "y" else allT[b][1]
        last = None
        for kc in range(n_ct):
            last = nc.sync.dma_start(
                out=dst[kc][:, ro : ro + rn],
                in_=srcb[b][ro : ro + rn, kc * P : (kc + 1) * P],
                transpose=True,
            )
        return last

    def cast_group(b, which, ro, rn):
        # bf16 [128, rn] tiles -> fp8 k-pair tiles, on DVE (idle engine).
        src = allT[b][0] if which == "y" else allT[b][1]
        dst = allT[b][2] if which == "y" else allT[b][3]
        for kc in range(n_ct):
            nc.vector.tensor_copy(
                dst[kc // 2][:, kc % 2, ro : ro + rn], src[kc][:, ro : ro + rn]
            )

    def pace(waiter, dependee):
        # Real semaphore edge: keeps the next copy-window out of flight until
        # the previous transpose-window drains (Tile serializes any transpose
        # against every in-flight copy, so un-paced casts stall transposes).
        if waiter is not None and dependee is not None:
            add_dep_helper(waiter.ins, dependee.ins, sync=True, reason="pace dma windows")

    # transpose-window: yT(b0) first half
    tg = transpose_group(0, "y", 0, half)
    cast_group(0, "y", 0, half)
    # copy-window: y0 second half + x0 first half casts
    xhalf = cfg.sq // 2
    if half < cfg.skv:
        c = nc.gpsimd.dma_start(out=yb[0][half:, :], in_=y[0][half:, :])
        pace(c, tg)
    c = nc.gpsimd.dma_start(out=xb[0][0:xhalf, :], in_=x[0][0:xhalf, :])
    pace(c, tg)
    # transpose-window: yT(b0) second half + xT(b0) first half together
    if half < cfg.skv:
        transpose_group(0, "y", half, cfg.skv - half)
        cast_group(0, "y", half, cfg.skv - half)
    tg = transpose_group(0, "x", 0, xhalf)
    cast_group(0, "x", 0, xhalf)
    # copy-window: x0 second half cast; then its transpose window
    c = nc.gpsimd.dma_start(out=xb[0][xhalf:, :], in_=x[0][xhalf:, :])
    pace(c, tg)
    tg = transpose_group(0, "x", xhalf, cfg.sq - xhalf)
    cast_group(0, "x", xhalf, cfg.sq - xhalf)
    allT["last_tg"] = tg
    # b1 chains are emitted inside the batch loop below (their windows land
    # under b0's scores/AV compute).

    for b in range(cfg.bl):
        yT, xT, yT8, xT8 = allT[b]
        if b > 0:
            c = nc.gpsimd.dma_start(out=yb[b][:], in_=y[b][:])
            pace(c, allT["last_tg"])
            tg = transpose_group(b, "y", 0, cfg.skv)
            cast_group(b, "y", 0, cfg.skv)
            c = nc.gpsimd.dma_start(out=xb[b][:], in_=x[b][:])
            pace(c, tg)
            pace(c, allT.get(f"wave_mid_{b - 1}_0"))
            tg = transpose_group(b, "x", 0, cfg.sq)
            cast_group(b, "x", 0, cfg.sq)
            allT["last_tg"] = tg

        # --- projections: kT/v aligned to the y halves, then qT ------------
        # kT/qT: [128, 2, S] k-pair tiles (DK=256 = one DoubleRow group).
        kT8 = pool.tile([P, 2, cfg.skv], fp8, tag="kT", name="kT8")
        qT8 = pool.tile([P, 2, cfg.sq], fp8, tag="qT", name="qT8")
        # v: k-pair tiles over the AV contraction (t): v8[g][:, j, :].
        v8 = [
            pool.tile([P, 2, cfg.e], fp8, tag="v", bufs=n_tg, name=f"v{g}")
            for g in range(n_tg)
        ]

        for ro, rn in y0_chunks:
            for no, nn_ in _chunks(rn, cfg.n_free):
                for md in range(n_dt):
                    ps = ps_mm.tile([P, cfg.n_free], f32, tag="mm", name="ps_p")
                    for g in range(n_cg):
                        nc.tensor.matmul(
                            ps[:, :nn_],
                            wk8[g][:, :, md * P : (md + 1) * P],
                            yT8[g][:, :, ro + no : ro + no + nn_],
                            start=(g == 0),
                            stop=(g == n_cg - 1),
                            perf_mode=DR,
                        )
                    nc.scalar.activation(
                        kT8[:, md, ro + no : ro + no + nn_], ps[:, :nn_], AF.Copy
                    )
            for mt in range(ro // P, (ro + rn) // P):
                for no, nn_ in _chunks(cfg.e, cfg.n_free):
                    ps = ps_mm.tile([P, cfg.n_free], f32, tag="mm", name="ps_v")
                    for g in range(n_cg):
                        nc.tensor.matmul(
                            ps[:, :nn_],
                            yT8[g][:, :, mt * P : (mt + 1) * P],
                            wv8[g][:, :, no : no + nn_],
                            start=(g == 0),
                            stop=(g == n_cg - 1),
                            perf_mode=DR,
                        )
                    nc.scalar.activation(
                        v8[mt // 2][:, mt % 2, no : no + nn_], ps[:, :nn_], AF.Copy
                    )
        for no, nn_ in _chunks(cfg.sq, cfg.n_free):
            for md in range(n_dt):
                ps = ps_mm.tile([P, cfg.n_free], f32, tag="mm", name="ps_q")
                for g in range(n_cg):
                    nc.tensor.matmul(
                        ps[:, :nn_],
                        wq8[g][:, :, md * P : (md + 1) * P],
                        xT8[g][:, :, no : no + nn_],
                        start=(g == 0),
                        stop=(g == n_cg - 1),
                        perf_mode=DR,
                    )
                nc.scalar.activation(
                    qT8[:, md, no : no + nn_], ps[:, :nn_], AF.Copy
                )

        # --- attention, one wave of s_block query columns at a time --------
        # Scores carry the 8*8 weight pre-scale; exp folds it back out.
        s_scale = cfg.scale / (WSC * WSC)
        for wo, wn in s_waves:
            # scoresT + exp: pT8[g][:, j, s_block] (t-tile 2g+j)
            pT8 = [
                pool.tile([P, 2, cfg.s_block], fp8, tag="pT", bufs=n_tg, name=f"pT{g}")
                for g in range(n_tg)
            ]
            for t in range(n_tt):
                for no, nn_ in _chunks(wn, cfg.n_free):
                    ps = ps_mm.tile([P, cfg.n_free], f32, tag="mm", name="ps_s")
                    nc.tensor.matmul(
                        ps[:, :nn_],
                        kT8[:, :, t * P : (t + 1) * P],
                        qT8[:, :, wo + no : wo + no + nn_],
                        start=True,
                        stop=True,
                        perf_mode=DR,
                    )
                    nc.scalar.activation(
                        pT8[t // 2][:, t % 2, no : no + nn_],
                        ps[:, :nn_],
                        AF.Exp,
                        scale=s_scale,
                    )

            # AV + rowsum + epilogue, per 128-row block of queries
            for mh in range(wn // P):
                sm = wo + mh * P  # global query row offset
                ps_e = ps_av.tile([P, cfg.e], f32, tag="av_e", name="ps_e")
                ps_sum = ps_av.tile([P, 1], f32, tag="av_s", name="ps_sum")
                e_chunks = _chunks(cfg.e, cfg.n_free)
                for g in range(n_tg):
                    lhsT = pT8[g][:, :, mh * P : (mh + 1) * P]
                    for no, nn_ in e_chunks:
                        nc.tensor.matmul(
                            ps_e[:, no : no + nn_],
                            lhsT,
                            v8[g][:, :, no : no + nn_],
                            start=(g == 0),
                            stop=(g == n_tg - 1),
                            perf_mode=DR,
                        )
                    nc.tensor.matmul(
                        ps_sum[:],
                        lhsT,
                        ones_col[:, :, 0:1],
                        start=(g == 0),
                        stop=(g == n_tg - 1),
                        perf_mode=DR,
                    )
                recip = pool.tile([P, 1], f32, tag="recip", bufs=8, name="recip")
                nc.vector.reciprocal(recip[:], ps_sum[:])
                # residual from the bf16 bounce: halves HBM vs re-reading x
                # fp32, and bf16 rounding (~0.2%) is far inside tolerance.
                xres = pool.tile([P, cfg.e], bf16, tag="xres", bufs=6, name="xres")
                nc.scalar.dma_start(out=xres[:], in_=xb[b][sm : sm + P, :])
                out_t = pool.tile([P, cfg.e], f32, tag="out_t", bufs=8, name="out_t")
                nc.vector.scalar_tensor_tensor(
                    out_t[:], ps_e[:], recip[:], xres[:], ALU.mult, ALU.add
                )
                st = nc.scalar.dma_start(out=out[b][sm : sm + P, :], in_=out_t[:])
                if mh == 1:
                    allT[f"wave_mid_{b}_{wo}"] = st
                allT[f"wave_end_{b}_{wo}"] = st

    ps_av.release()
    ps_mm.release()
    pool.release()


def make_tile_kernel(cfg):
    """Adapter with the (tc, outs, ins) signature used by run_kernel/test.py."""

    def k(tc, outs, ins):
        emit_cross_attention(tc, outs, ins, cfg)

    return k


def _build(cfg):
    import concourse.bacc as bacc
    import concourse.mybir as mybir
    import concourse.tile as tile

    f32 = mybir.dt.float32
    nc = bacc.Bacc(
        "TRN2",
        target_bir_lowering=False,
        debug=False,
        enable_asserts=False,
        num_devices=N_CORES,
    )
    ins = {
        "x": nc.dram_tensor("x", [cfg.bl, cfg.sq, cfg.c], f32, kind="ExternalInput").ap(),
        "y": nc.dram_tensor("y", [cfg.bl, cfg.skv, cfg.c], f32, kind="ExternalInput").ap(),
        "Wq": nc.dram_tensor("Wq", [cfg.c, cfg.dk], f32, kind="ExternalInput").ap(),
        "Wk": nc.dram_tensor("Wk", [cfg.c, cfg.dk], f32, kind="ExternalInput").ap(),
        "Wv": nc.dram_tensor("Wv", [cfg.c, cfg.e], f32, kind="ExternalInput").ap(),
    }
    outs = {
        "out": nc.dram_tensor("out", [cfg.bl, cfg.sq, cfg.e], f32, kind="ExternalOutput").ap()
    }
    with tile.TileContext(nc) as tc:
        emit_cross_attention(tc, outs, ins, cfg)
    nc.compile()
    return nc


_CACHED = {}


def run_on_cores(x, y, Wq, Wk, Wv, trace=False):
    from concourse import bass_utils

    cfg = CFG(B_FULL // N_CORES, S_Q, S_KV, C_DIM, DK, E_DIM)
    key = "full"
    if key not in _CACHED:
        _CACHED[key] = _build(cfg)
    nc = _CACHED[key]

    bl = cfg.bl
    in_maps = [
        {
            "x": np.ascontiguousarray(x[i * bl : (i + 1) * bl]),
            "y": np.ascontiguousarray(y[i * bl : (i + 1) * bl]),
            "Wq": Wq,
            "Wk": Wk,
            "Wv": Wv,
        }
        for i in range(N_CORES)
    ]
    res = bass_utils.run_bass_kernel_spmd(
        nc, in_maps, core_ids=list(range(N_CORES)), trace=trace
    )
    out = np.concatenate([r["out"] for r in res.results], axis=0)
    return out, res


def kernel(x, y, Wq, Wk, Wv):
    x = np.asarray(x, dtype=np.float32)
    y = np.asarray(y, dtype=np.float32)
    Wq = np.asarray(Wq, dtype=np.float32)
    Wk = np.asarray(Wk, dtype=np.float32)
    Wv = np.asarray(Wv, dtype=np.float32)
    out, _ = run_on_cores(x, y, Wq, Wk, Wv, trace=False)
    return out


# revision 9
# speedup vs baseline: 1.2570x; 1.1513x over previous
"""Cross-attention Trainium2 kernel (8 NeuronCores, batch-data-parallel).

Computes, per batch element b:
    q = x[b] @ Wq            [S, DK]
    k = y[b] @ Wk            [S, DK]
    v = y[b] @ Wv            [S, E]
    p = exp((q @ k.T) / sqrt(E))        (no max-subtraction: logits ~ N(0, .25))
    out[b] = (p @ v) / rowsum(p) + x[b]

Layout strategy (per core, BL=2 batches):
  - All matmuls run in fp8e4 with perf_mode=DoubleRow (2 fp8 weights/cell,
    2 MACs/cycle): operands are stored as [128, 2, free] "k-pair" tiles so a
    single matmul contracts 256 elements; PSUM accumulates fp32.
  - Weights are pre-scaled by 8 when cast to fp8 (keeps N(0,1/1024) entries
    out of the fp8 subnormal range); the score scale folds the 8*8 back out,
    and the rowsum ones-column is 8.0 so the softmax normalization of the
    8x-scaled v cancels exactly.
  - Activations are transposed on-chip (cast-DMA fp32->bf16 into a DRAM
    bounce, then xbar DMA-transpose into SBUF bf16, then DVE cast to the
    fp8 k-pair tiles) so the contraction dim of every matmul sits on
    partitions:
        xT, yT : [C, S]     qT = Wq.T @ xT : [DK, S]   kT : [DK, S]
        v  = yT.T @ Wv : [S_kv, E]  (natural layout)
        sT = kT.T @ qT : [S_kv, S_q]   (scoresT; softmax axis = partitions)
        pT = exp(sT*scale)             (stationary of the AV matmul)
        out = pT.T @ [v | 8]           (8s column yields 8*rowsum(p) free)
  - Epilogue fuses (psum * 1/rowsum8) + x in one DVE scalar_tensor_tensor.
"""

import math

import numpy as np

# Full-problem constants (hardcoded per the harness contract).
B_FULL = 16
N_CORES = 8
S_Q = 2048
S_KV = 2048
C_DIM = 1024  # input feature dim (contraction of the projections)
DK = 256  # q/k head dim
E_DIM = 1024  # v / output dim
P = 128
WSC = 8.0  # fp8 pre-scale on Wq/Wk/Wv (and the rowsum ones column)


class CFG:
    def __init__(self, bl, sq, skv, c, dk, e, s_block=None, n_free=512):
        assert sq % P == 0 and skv % P == 0 and c % P == 0 and dk % P == 0
        self.bl = bl  # batches per core
        self.sq = sq
        self.skv = skv
        self.c = c
        self.dk = dk
        self.e = e
        self.s_block = s_block or min(1024, sq)  # query cols processed per wave
        assert sq % self.s_block == 0
        self.n_free = n_free  # moving-operand free-dim per matmul
        self.scale = 1.0 / math.sqrt(e)


def _chunks(total, size):
    out = []
    o = 0
    while o < total:
        out.append((o, min(size, total - o)))
        o += size
    return out


def emit_cross_attention(tc, outs, ins, cfg):
    """Emit the kernel into TileContext `tc`.

    outs/ins are dicts of DRAM APs: ins = x, y, Wq, Wk, Wv ; outs = out.
    x/y/out: [bl, sq|skv, c|e] fp32. Weights: [c, dk|e] fp32.
    """
    import concourse.mybir as mybir
    from concourse.mybir import ActivationFunctionType as AF
    from concourse.mybir import AluOpType as ALU
    from concourse.tile_rust import add_dep_helper

    DR = mybir.MatmulPerfMode.DoubleRow

    nc = tc.nc
    bf16 = mybir.dt.bfloat16
    fp8 = mybir.dt.float8e4
    f32 = mybir.dt.float32

    x, y, Wq, Wk, Wv = ins["x"], ins["y"], ins["Wq"], ins["Wk"], ins["Wv"]
    out = outs["out"]

    n_ct = cfg.c // P  # 128-contraction tiles of the projections
    n_cg = n_ct // 2  # DoubleRow (256-contraction) groups of the projections
    n_tt = cfg.skv // P  # key/value 128-tiles
    n_tg = n_tt // 2  # key/value DoubleRow groups (AV contraction)
    n_dt = cfg.dk // P  # qk-dim 128-tiles (score contraction; must be 2)
    assert n_dt == 2, "scores assume DK == 256 (one DoubleRow group)"
    s_waves = _chunks(cfg.sq, cfg.s_block)

    # DRAM bounce buffers for the bf16 copies of x and y (per local batch).
    xb = nc.dram_tensor("xb16", [cfg.bl, cfg.sq, cfg.c], bf16).ap()
    yb = nc.dram_tensor("yb16", [cfg.bl, cfg.skv, cfg.c], bf16).ap()

    pool = tc.alloc_tile_pool(name="main", bufs=1)
    ps_mm = tc.alloc_tile_pool(name="ps_mm", bufs=2, space="PSUM")
    ps_av = tc.alloc_tile_pool(name="ps_av", bufs=2, space="PSUM")

    # Measured DMA facts this layout is built on:
    #   - SWDGE D2D cast runs at ~360 GB/s payload.
    #   - xbar transposes cost ~1us fixed + ~400 GB/s; they only exist on
    #     one ring (concurrent transposes on both HWDGE rings corrupt), and
    #     Tile serializes every transpose group against ALL in-flight DMAs.
    #     So the global stream alternates copy-windows and transpose-windows,
    #     ordered here so each window's data is needed just after it closes.
    #   - SWDGE queue: casts only; sync ring: transposes only; scalar ring:
    #     weights / residual / output plain DMAs.
    half = cfg.skv // 2 if cfg.skv >= 1024 else cfg.skv
    y0_chunks = _chunks(cfg.skv, half)

    # fp8 k-pair weight tiles: w8[g][:, j, :] holds rows (2g+j)*128..+128.
    wq8 = [pool.tile([P, 2, cfg.dk], fp8, tag=f"wq{g}", name=f"wq{g}") for g in range(n_cg)]
    wk8 = [pool.tile([P, 2, cfg.dk], fp8, tag=f"wk{g}", name=f"wk{g}") for g in range(n_cg)]
    wv8 = [pool.tile([P, 2, cfg.e], fp8, tag=f"wv{g}", name=f"wv{g}") for g in range(n_cg)]

    def load_weight(which, w_dram, w_tiles, wdim, kc):
        w_f = pool.tile([P, cfg.e], f32, tag="wstage", bufs=2, name=f"wf{which}{kc}")
        nc.scalar.dma_start(out=w_f[:, :wdim], in_=w_dram[kc * P : (kc + 1) * P, :])
        nc.vector.tensor_scalar_mul(w_tiles[kc // 2][:, kc % 2, :], w_f[:, :wdim], WSC)

    # copy-window 0: y0 first half cast (SWDGE) + wk loads (scalar ring)
    nc.gpsimd.dma_start(out=yb[0][0:half, :], in_=y[0][0:half, :])
    for kc in range(n_ct):
        load_weight("k", Wk, wk8, cfg.dk, kc)

    # 8.0 column (fp8 exact): rowsum of p gets the same 8x scale as v.
    ones_col = pool.tile([P, 2, 16], fp8, tag="ones", name="ones")
    nc.gpsimd.memset(ones_col[:], WSC)

    allT = {}
    for b in range(cfg.bl):
        yT = []
        xT = []
        for kc in range(n_ct):
            yT_t = pool.tile([P, cfg.skv], bf16, tag="actT", bufs=2 * n_ct, name=f"yT{kc}")
            yT.append(yT_t)
        for kc in range(n_ct):
            xT_t = pool.tile([P, cfg.sq], bf16, tag="actT", bufs=2 * n_ct, name=f"xT{kc}")
            xT.append(xT_t)
        # fp8 k-pair copies of the transposed activations.
        yT8 = [
            pool.tile([P, 2, cfg.skv], fp8, tag="actT8", bufs=n_ct, name=f"yT8_{g}")
            for g in range(n_cg)
        ]
        xT8 = [
            pool.tile([P, 2, cfg.sq], fp8, tag="actT8", bufs=n_ct, name=f"xT8_{g}")
            for g in range(n_cg)
        ]
        allT[b] = (yT, xT, yT8, xT8)

    def transpose_group(b, which, ro, rn):
        srcb = yb if which == "y" else xb
        dst = allT[b][0] if which == "y" else allT[b][1]
        last = None
        for kc in range(n_ct):
            last = nc.sync.dma_start(
                out=dst[kc][:, ro : ro + rn],
                in_=srcb[b][ro : ro + rn, kc * P : (kc + 1) * P],
                transpose=True,
            )
        return last

    def cast_group(b, which, ro, rn):
        # bf16 [128, rn] tiles -> fp8 k-pair tiles, on DVE (idle engine).
        src = allT[b][0] if which == "y" else allT[b][1]
        dst = allT[b][2] if which == "y" else allT[b][3]
        for kc in range(n_ct):
            nc.vector.tensor_copy(
                dst[kc // 2][:, kc % 2, ro : ro + rn], src[kc][:, ro : ro + rn]
            )

    def pace(waiter, dependee):
        # Real semaphore edge: keeps the next copy-window out of flight until
        # the previous transpose-window drains (Tile serializes any transpose
        # against every in-flight copy, so un-paced casts stall transposes).
        if waiter is not None and dependee is not None:
            add_dep_helper(waiter.ins, dependee.ins, sync=True, reason="pace dma windows")

    # transpose-window: yT(b0) first half
    tg = transpose_group(0, "y", 0, half)
    cast_group(0, "y", 0, half)
    # copy-window: y0 second half + wv, wq loads
    if half < cfg.skv:
        c = nc.gpsimd.dma_start(out=yb[0][half:, :], in_=y[0][half:, :])
        pace(c, tg)
    for kc in range(n_ct):
        load_weight("v", Wv, wv8, cfg.e, kc)
    for kc in range(n_ct):
        load_weight("q", Wq, wq8, cfg.dk, kc)
    if half < cfg.skv:
        tg = transpose_group(0, "y", half, cfg.skv - half)
        cast_group(0, "y", half, cfg.skv - half)
    # copy-window: x0 cast; then xT(b0) transposes
    c = nc.gpsimd.dma_start(out=xb[0][:], in_=x[0][:])
    pace(c, tg)
    tg = transpose_group(0, "x", 0, cfg.sq)
    cast_group(0, "x", 0, cfg.sq)
    allT["last_tg"] = tg
    # b1 chains are emitted inside the batch loop below (their windows land
    # under b0's scores/AV compute).

    for b in range(cfg.bl):
        yT, xT, yT8, xT8 = allT[b]
        if b > 0:
            c = nc.gpsimd.dma_start(out=yb[b][:], in_=y[b][:])
            pace(c, allT["last_tg"])
            tg = transpose_group(b, "y", 0, cfg.skv)
            cast_group(b, "y", 0, cfg.skv)
            c = nc.gpsimd.dma_start(out=xb[b][:], in_=x[b][:])
            pace(c, tg)
            pace(c, allT.get(f"wave_mid_{b - 1}_0"))
            tg = transpose_group(b, "x", 0, cfg.sq)
            cast_group(b, "x", 0, cfg.sq)
            allT["last_tg"] = tg

        # --- projections: kT/v aligned to the y halves, then qT ------------
        # kT/qT: [128, 2, S] k-pair tiles (DK=256 = one DoubleRow group).
        kT8 = pool.tile([P, 2, cfg.skv], fp8, tag="kT", name="kT8")
        qT8 = pool.tile([P, 2, cfg.sq], fp8, tag="qT", name="qT8")
        # v: k-pair tiles over the AV contraction (t): v8[g][:, j, :].
        v8 = [
            pool.tile([P, 2, cfg.e], fp8, tag="v", bufs=n_tg, name=f"v{g}")
            for g in range(n_tg)
        ]

        for ro, rn in y0_chunks:
            for no, nn_ in _chunks(rn, cfg.n_free):
                for md in range(n_dt):
                    ps = ps_mm.tile([P, cfg.n_free], f32, tag="mm", name="ps_p")
                    for g in range(n_cg):
                        nc.tensor.matmul(
                            ps[:, :nn_],
                            wk8[g][:, :, md * P : (md + 1) * P],
                            yT8[g][:, :, ro + no : ro + no + nn_],
                            start=(g == 0),
                            stop=(g == n_cg - 1),
                            perf_mode=DR,
                        )
                    nc.scalar.activation(
                        kT8[:, md, ro + no : ro + no + nn_], ps[:, :nn_], AF.Copy
                    )
            for mt in range(ro // P, (ro + rn) // P):
                for no, nn_ in _chunks(cfg.e, cfg.n_free):
                    ps = ps_mm.tile([P, cfg.n_free], f32, tag="mm", name="ps_v")
                    for g in range(n_cg):
                        nc.tensor.matmul(
                            ps[:, :nn_],
                            yT8[g][:, :, mt * P : (mt + 1) * P],
                            wv8[g][:, :, no : no + nn_],
                            start=(g == 0),
                            stop=(g == n_cg - 1),
                            perf_mode=DR,
                        )
                    nc.scalar.activation(
                        v8[mt // 2][:, mt % 2, no : no + nn_], ps[:, :nn_], AF.Copy
                    )
        for no, nn_ in _chunks(cfg.sq, cfg.n_free):
            for md in range(n_dt):
                ps = ps_mm.tile([P, cfg.n_free], f32, tag="mm", name="ps_q")
                for g in range(n_cg):
                    nc.tensor.matmul(
                        ps[:, :nn_],
                        wq8[g][:, :, md * P : (md + 1) * P],
                        xT8[g][:, :, no : no + nn_],
                        start=(g == 0),
                        stop=(g == n_cg - 1),
                        perf_mode=DR,
                    )
                nc.scalar.activation(
                    qT8[:, md, no : no + nn_], ps[:, :nn_], AF.Copy
                )

        # --- attention, one wave of s_block query columns at a time --------
        # Scores carry the 8*8 weight pre-scale; exp folds it back out.
        s_scale = cfg.scale / (WSC * WSC)
        for wo, wn in s_waves:
            # scoresT + exp: pT8[g][:, j, s_block] (t-tile 2g+j)
            pT8 = [
                pool.tile([P, 2, cfg.s_block], fp8, tag="pT", bufs=n_tg, name=f"pT{g}")
                for g in range(n_tg)
            ]
            for t in range(n_tt):
                for no, nn_ in _chunks(wn, cfg.n_free):
                    ps = ps_mm.tile([P, cfg.n_free], f32, tag="mm", name="ps_s")
                    nc.tensor.matmul(
                        ps[:, :nn_],
                        kT8[:, :, t * P : (t + 1) * P],
                        qT8[:, :, wo + no : wo + no + nn_],
                        start=True,
                        stop=True,
                        perf_mode=DR,
                    )
                    nc.scalar.activation(
                        pT8[t // 2][:, t % 2, no : no + nn_],
                        ps[:, :nn_],
                        AF.Exp,
                        scale=s_scale,
                    )

            # AV + rowsum + epilogue, per 128-row block of queries
            for mh in range(wn // P):
                sm = wo + mh * P  # global query row offset
                ps_e = ps_av.tile([P, cfg.e], f32, tag="av_e", name="ps_e")
                ps_sum = ps_av.tile([P, 1], f32, tag="av_s", name="ps_sum")
                e_chunks = _chunks(cfg.e, cfg.n_free)
                for g in range(n_tg):
                    lhsT = pT8[g][:, :, mh * P : (mh + 1) * P]
                    for no, nn_ in e_chunks:
                        nc.tensor.matmul(
                            ps_e[:, no : no + nn_],
                            lhsT,
                            v8[g][:, :, no : no + nn_],
                            start=(g == 0),
                            stop=(g == n_tg - 1),
                            perf_mode=DR,
                        )
                    nc.tensor.matmul(
                        ps_sum[:],
                        lhsT,
                        ones_col[:, :, 0:1],
                        start=(g == 0),
                        stop=(g == n_tg - 1),
                        perf_mode=DR,
                    )
                recip = pool.tile([P, 1], f32, tag="recip", bufs=8, name="recip")
                nc.vector.reciprocal(recip[:], ps_sum[:])
                # residual from the bf16 bounce: halves HBM vs re-reading x
                # fp32, and bf16 rounding (~0.2%) is far inside tolerance.
                xres = pool.tile([P, cfg.e], bf16, tag="xres", bufs=6, name="xres")
                nc.scalar.dma_start(out=xres[:], in_=xb[b][sm : sm + P, :])
                out_t = pool.tile([P, cfg.e], f32, tag="out_t", bufs=8, name="out_t")
                nc.vector.scalar_tensor_tensor(
                    out_t[:], ps_e[:], recip[:], xres[:], ALU.mult, ALU.add
                )
                st = nc.scalar.dma_start(out=out[b][sm : sm + P, :], in_=out_t[:])
                if mh == 1:
                    allT[f"wave_mid_{b}_{wo}"] = st
                allT[f"wave_end_{b}_{wo}"] = st

    ps_av.release()
    ps_mm.release()
    pool.release()


def make_tile_kernel(cfg):
    """Adapter with the (tc, outs, ins) signature used by run_kernel/test.py."""

    def k(tc, outs, ins):
        emit_cross_attention(tc, outs, ins, cfg)

    return k


def _build(cfg):
    import concourse.bacc as bacc
    import concourse.mybir as mybir
    import concourse.tile as tile

    f32 = mybir.dt.float32
    nc = bacc.Bacc(
        "TRN2",
        target_bir_lowering=False,
        debug=False,
        enable_asserts=False,
        num_devices=N_CORES,
    )
    ins = {
        "x": nc.dram_tensor("x", [cfg.bl, cfg.sq, cfg.c], f32, kind="ExternalInput").ap(),
        "y": nc.dram_tensor("y", [cfg.bl, cfg.skv, cfg.c], f32, kind="ExternalInput").ap(),
        "Wq": nc.dram_tensor("Wq", [cfg.c, cfg.dk], f32, kind="ExternalInput").ap(),
        "Wk": nc.dram_tensor("Wk", [cfg.c, cfg.dk], f32, kind="ExternalInput").ap(),
        "Wv": nc.dram_tensor("Wv", [cfg.c, cfg.e], f32, kind="ExternalInput").ap(),
    }
    outs = {
        "out": nc.dram_tensor("out", [cfg.bl, cfg.sq, cfg.e], f32, kind="ExternalOutput").ap()
    }
    with tile.TileContext(nc) as tc:
        emit_cross_attention(tc, outs, ins, cfg)
    nc.compile()
    return nc


_CACHED = {}


def run_on_cores(x, y, Wq, Wk, Wv, trace=False):
    from concourse import bass_utils

    cfg = CFG(B_FULL // N_CORES, S_Q, S_KV, C_DIM, DK, E_DIM)
    key = "full"
    if key not in _CACHED:
        _CACHED[key] = _build(cfg)
    nc = _CACHED[key]

    bl = cfg.bl
    in_maps = [
        {
            "x": np.ascontiguousarray(x[i * bl : (i + 1) * bl]),
            "y": np.ascontiguousarray(y[i * bl : (i + 1) * bl]),
            "Wq": Wq,
            "Wk": Wk,
            "Wv": Wv,
        }
        for i in range(N_CORES)
    ]
    res = bass_utils.run_bass_kernel_spmd(
        nc, in_maps, core_ids=list(range(N_CORES)), trace=trace
    )
    out = np.concatenate([r["out"] for r in res.results], axis=0)
    return out, res


def kernel(x, y, Wq, Wk, Wv):
    x = np.asarray(x, dtype=np.float32)
    y = np.asarray(y, dtype=np.float32)
    Wq = np.asarray(Wq, dtype=np.float32)
    Wk = np.asarray(Wk, dtype=np.float32)
    Wv = np.asarray(Wv, dtype=np.float32)
    out, _ = run_on_cores(x, y, Wq, Wk, Wv, trace=False)
    return out
